# revision 6
# baseline (speedup 1.0000x reference)
"""Causal single-head attention (B=4, S=2048, D=768) on 8 trn2 NeuronCores.

Sharding: batch (4) x role (2). Core c = 2*b + r handles batch b; role r owns
query blocks {2i+r} and seq-half r of the K/V projections. The pair exchanges
K/V halves via a 2-rank AllGather, so each core projects Q(1024 rows) +
K(half) + V(half) instead of duplicating full K/V.

All device compute is bf16 (fp32 PSUM accumulation); final output fp32.

Attention is computed in transposed-score layout (ST[k, q] = K_j @ Q^T), which
removes the PE transposes of the baseline: for each key block j we compute
ST tiles over the q-block prefix (q blocks stored in descending order), apply
the causal/padding mask to the last 128 columns only (mask content is
role-specific DATA, so the instruction stream is SPMD-identical), exp to
PT_j in SBUF, then accumulate O[q] = sum_j PT_j[:, q].T @ V_j. Row softmax
sums come for free from a ones-column appended to V.
"""

import os
import sys

for _p in ("/opt/trn_rl_repo", "/root/.axon_site/_ro/trn_rl_repo"):
    if os.path.isdir(_p) and _p not in sys.path:
        sys.path.append(_p)

import numpy as np

import concourse.bacc as bacc
import concourse.mybir as mybir
import concourse.tile as tile
from concourse._compat import get_trn_type

B, S, D = 4, 2048, 768
P = 128
DC = D // P          # 6 contraction / dout chunks
SB = S // P          # 16 seq blocks
NQ = 8               # q-blocks per core
QW = NQ * P          # 1024 q rows per core
H = S // 2           # seq half owned per core for K/V
SCALE = 1.0 / float(np.sqrt(D))
MASK_VAL = -1e30

F32 = mybir.dt.float32
BF16 = mybir.dt.bfloat16

# role r owns q blocks {2i+r}, listed descending (prefix property for ST tiles)
QSETS = ([14, 12, 10, 8, 6, 4, 2, 0], [15, 13, 11, 9, 7, 5, 3, 1])
# padded q-prefix width (in blocks) of the ST tile for key block j
PJ = [8, 8, 7, 7, 6, 6, 5, 5, 4, 4, 3, 3, 2, 2, 1, 1]
# padded key extent (in blocks) of the O accumulation for q slot p
EP = [16, 14, 12, 10, 8, 6, 4, 2]
CC_GROUPS = [[0, 1], [2, 3], [4, 5], [6, 7]]

COLLECTIVE = True


def build_nc(reps=1, collective=None):
    collective = COLLECTIVE if collective is None else collective
    nc = bacc.Bacc(
        get_trn_type() or "TRN2",
        target_bir_lowering=False,
        debug=False,
        num_devices=8,
        dynamic_dma_scratch_size=2048,
    )
    xt_cols = H if collective else S
    xt_d = nc.dram_tensor("xt", [D, xt_cols], BF16, kind="ExternalInput").ap()
    xtq_d = nc.dram_tensor("xtq", [D, QW], BF16, kind="ExternalInput").ap()
    wq_d = nc.dram_tensor("wq", [D, D], BF16, kind="ExternalInput").ap()
    wk_d = nc.dram_tensor("wk", [D, D], BF16, kind="ExternalInput").ap()
    wv_d = nc.dram_tensor("wv", [D, D], BF16, kind="ExternalInput").ap()
    mask_d = nc.dram_tensor("mask", [SB, P, P], F32, kind="ExternalInput").ap()
    ones_d = nc.dram_tensor("ones", [P, 1], BF16, kind="ExternalInput").ap()
    o_d = nc.dram_tensor("o", [QW, D], F32, kind="ExternalOutput").ap()

    for _rep in range(reps):
        _emit_body(nc, xt_d, xtq_d, wq_d, wk_d, wv_d, mask_d, ones_d, o_d,
                   collective=collective)
    return nc


def _emit_body(nc, xt_d, xtq_d, wq_d, wk_d, wv_d, mask_d, ones_d, o_d,
               collective=True):
    xt_cols = H if collective else S
    with tile.TileContext(nc) as tc:
        persist = tc.alloc_tile_pool(name="persist", bufs=1)
        kt = [persist.tile([P, S], BF16, tag=f"kt{c}", name=f"kt{c}") for c in range(DC)]
        v = [persist.tile([P, D + 1], BF16, tag=f"v{j}", name=f"v{j}") for j in range(SB)]
        qt = [persist.tile([P, QW], BF16, tag=f"qt{c}", name=f"qt{c}") for c in range(DC)]
        pt = [persist.tile([P, PJ[j] * P], BF16, tag=f"pt{j}", name=f"pt{j}")
              for j in range(SB)]
        masks = [persist.tile([P, P], F32, tag=f"mask{j}", name=f"mask{j}")
                 for j in range(SB)]
        for j in range(SB):
            nc.scalar.dma_start(masks[j][:], mask_d[j])
        for j in range(SB):
            nc.scalar.dma_start(v[j][:, D:D + 1], ones_d[:])

        if collective:
            dram = tc.alloc_tile_pool(name="dram", bufs=1, space="DRAM")
            kstage_d = dram.tile([D, H], BF16, name="kstage")
            kgath_d = dram.tile([2 * D, H], BF16, name="kgath")
            vstage_d = dram.tile([H, D], BF16, name="vstage")
            vgath_d = dram.tile([S, D], BF16, name="vgath")

        # all external inputs requested up-front so later (collective-gated)
        # readback DMAs can't head-of-line-block them on the SP queue
        win = tc.alloc_tile_pool(name="win", bufs=1)
        wk = [win.tile([P, D], BF16, tag=f"wk{c}", name=f"wk{c}") for c in range(DC)]
        wv = [win.tile([P, D], BF16, tag=f"wv{c}", name=f"wv{c}") for c in range(DC)]
        wq = [win.tile([P, D], BF16, tag=f"wq{c}", name=f"wq{c}") for c in range(DC)]
        xtq = [win.tile([P, QW], BF16, tag=f"xtq{c}", name=f"xtq{c}")
               for c in range(DC)]
        for c in range(DC):
            nc.sync.dma_start(wk[c][:], wk_d[c * P:(c + 1) * P, :])
        for c in range(DC):
            nc.sync.dma_start(wv[c][:], wv_d[c * P:(c + 1) * P, :])
        for c in range(DC):
            nc.sync.dma_start(xtq[c][:], xtq_d[c * P:(c + 1) * P, :])
            nc.sync.dma_start(wq[c][:], wq_d[c * P:(c + 1) * P, :])

        xt_pool = tc.alloc_tile_pool(name="xt_pool", bufs=1)
        xt = [xt_pool.tile([P, xt_cols], BF16, tag=f"xt{c}", name=f"xt{c}")
              for c in range(DC)]
        for c in range(DC):
            nc.scalar.dma_start(xt[c][:], xt_d[c * P:(c + 1) * P, :])

        # ---- K projection over own seq half (full seq if no collective)
        with (tc.tile_pool(name="stage_k", bufs=1) as sk,
              tc.tile_pool(name="psum_k", bufs=4, space="PSUM") as ppk):
            for co in range(DC):
                for g in range(xt_cols // 512):
                    ps = ppk.tile([P, 512], F32, tag="pp", name="pp")
                    for ci in range(DC):
                        nc.tensor.matmul(
                            ps[:],
                            wk[ci][:, co * P:(co + 1) * P],
                            xt[ci][:, g * 512:(g + 1) * 512],
                            start=(ci == 0), stop=(ci == DC - 1),
                        )
                    if collective:
                        st = sk.tile([P, 512], BF16, tag="kst", name="kst", bufs=4)
                        nc.scalar.copy(st[:], ps[:])
                        nc.scalar.dma_start(
                            kstage_d[co * P:(co + 1) * P, g * 512:(g + 1) * 512],
                            st[:])
                    else:
                        nc.scalar.copy(kt[co][:, g * 512:(g + 1) * 512], ps[:])
        if collective:
            nc.gpsimd.collective_compute(
                "AllGather", mybir.AluOpType.bypass,
                replica_groups=CC_GROUPS,
                ins=[kstage_d[:]], outs=[kgath_d[:]],
            )
            for c in range(DC):
                nc.sync.dma_start(kt[c][:, 0:H], kgath_d[c * P:(c + 1) * P, :])
                nc.sync.dma_start(kt[c][:, H:S],
                                  kgath_d[D + c * P:D + (c + 1) * P, :])

        # ---- V projection over own seq half
        with tc.tile_pool(name="psum_v", bufs=3, space="PSUM") as ppv:
            for jl in range(xt_cols // P):
                ps = ppv.tile([P, D], F32, tag="ppv", name="ppv")
                for (n0, nw) in ((0, 512), (512, 256)):
                    for ci in range(DC):
                        nc.tensor.matmul(
                            ps[:, n0:n0 + nw],
                            xt[ci][:, jl * P:(jl + 1) * P],
                            wv[ci][:, n0:n0 + nw],
                            start=(ci == 0), stop=(ci == DC - 1),
                        )
                if collective:
                    st = win.tile([P, D], BF16, tag="vst", name="vst", bufs=3)
                    nc.vector.tensor_copy(st[:], ps[:])
                    nc.scalar.dma_start(vstage_d[jl * P:(jl + 1) * P, :], st[:])
                else:
                    nc.vector.tensor_copy(v[jl][:, 0:D], ps[:])
        if collective:
            nc.gpsimd.collective_compute(
                "AllGather", mybir.AluOpType.bypass,
                replica_groups=CC_GROUPS,
                ins=[vstage_d[:]], outs=[vgath_d[:]],
            )
            for j in range(SB):
                nc.sync.dma_start(v[j][:, 0:D], vgath_d[j * P:(j + 1) * P, :])
        xt_pool.release()

        # ---- Q projection (this core's 1024 q rows, descending-block order)
        with tc.tile_pool(name="psum_q", bufs=4, space="PSUM") as ppq:
            for co in range(DC):
                for g in range(QW // 512):
                    ps = ppq.tile([P, 512], F32, tag="pp", name="pp")
                    for ci in range(DC):
                        nc.tensor.matmul(
                            ps[:],
                            wq[ci][:, co * P:(co + 1) * P],
                            xtq[ci][:, g * 512:(g + 1) * 512],
                            start=(ci == 0), stop=(ci == DC - 1),
                        )
                    nc.scalar.copy(qt[co][:, g * 512:(g + 1) * 512], ps[:])
        win.release()

        _emit_attention(nc, tc, qt, kt, v, pt, masks, o_d)

        if collective:
            dram.release()
        persist.release()


def _emit_attention(nc, tc, qt, kt, v, pt, masks, o_d):
    with (
        tc.tile_pool(name="psum_a", bufs=2, space="PSUM") as psa,
        tc.tile_pool(name="psum_b", bufs=2, space="PSUM") as psb,
        tc.tile_pool(name="o_sb", bufs=2) as o_pool,
        tc.tile_pool(name="small", bufs=4) as small,
    ):
        # Phase A: ST_j = KT_j.T @ QT prefix -> mask last block -> exp -> PT_j
        for j in range(SB):
            W = PJ[j] * P
            ps = psa.tile([P, 1024], F32, tag="sa", name="sa")
            grps = ((0, 512), (512, W - 512)) if W > 512 else ((0, W),)
            for (c0, cw) in grps:
                for ci in range(DC):
                    nc.tensor.matmul(
                        ps[:, c0:c0 + cw],
                        kt[ci][:, j * P:(j + 1) * P],
                        qt[ci][:, c0:c0 + cw],
                        start=(ci == 0), stop=(ci == DC - 1),
                    )
            off = W - P
            nc.vector.tensor_add(ps[:, off:off + P], ps[:, off:off + P],
                                 masks[j][:])
            nc.scalar.activation(
                pt[j][:], ps[:, :W],
                mybir.ActivationFunctionType.Exp,
                scale=SCALE,
            )

        # Phase B: O_p = sum_j PT_j[:, p].T @ V_j ; last V col = ones -> rowsum
        for p in range(NQ - 1, -1, -1):
            e = EP[p]
            po = psb.tile([P, D + 1], F32, tag="po", name="po")
            for j in range(e):
                lhs = pt[j][:, p * P:(p + 1) * P]
                for (n0, nw) in ((0, 512), (512, 257)):
                    nc.tensor.matmul(
                        po[:, n0:n0 + nw],
                        lhs,
                        v[j][:, n0:n0 + nw],
                        start=(j == 0), stop=(j == e - 1),
                    )
            rec = small.tile([P, 1], F32, tag="rec", name="rec")
            nc.vector.reciprocal(rec[:], po[:, D:D + 1])
            osb = o_pool.tile([P, D], F32, tag="osb", name="osb")
            nc.scalar.activation(
                osb[:], po[:, 0:D],
                mybir.ActivationFunctionType.Copy,
                scale=rec[:, 0:1],
            )
            nc.sync.dma_start(o_d[p * P:(p + 1) * P, :], osb[:])


# ---------------------------------------------------------------------------
# host side

def _build_masks():
    """masks[r] : [16,128,128] additive mask applied to the LAST 128 columns
    of ST tile j. Role r: diag block (j%2==r) -> transposed tril; role 0 odd j
    -> all -1e30 (kills the padding column later read by phase B); else 0."""
    col = np.arange(P)[None, :]
    row = np.arange(P)[:, None]
    tril = np.where(col >= row, 0.0, MASK_VAL).astype(np.float32)
    out = []
    for r in (0, 1):
        m = np.zeros((SB, P, P), np.float32)
        for j in range(SB):
            if j % 2 == r:
                m[j] = tril
            elif r == 0:
                m[j] = MASK_VAL
        out.append(m)
    return out


_STATE = {}


def _get_nc(collective=None):
    collective = COLLECTIVE if collective is None else collective
    key = f"nc_cc{int(collective)}"
    if key not in _STATE:
        nc = build_nc(collective=collective)
        nc.finalize()
        _STATE[key] = nc
    return _STATE[key]


def build_in_maps(x, Wq, Wk, Wv, collective=None):
    import ml_dtypes
    bf16 = ml_dtypes.bfloat16
    collective = COLLECTIVE if collective is None else collective
    masks = _build_masks()
    ones = np.ones((P, 1), bf16)
    wq16 = np.ascontiguousarray(np.asarray(Wq).astype(bf16))
    wk16 = np.ascontiguousarray(np.asarray(Wk).astype(bf16))
    wv16 = np.ascontiguousarray(np.asarray(Wv).astype(bf16))
    in_maps = []
    for b in range(B):
        xt = np.asarray(x[b]).T.astype(bf16)          # [768, 2048]
        for r in (0, 1):
            cols = np.concatenate(
                [xt[:, g * P:(g + 1) * P] for g in QSETS[r]], axis=1)
            xt_in = xt[:, r * H:(r + 1) * H] if collective else xt
            in_maps.append({
                "xt": np.ascontiguousarray(xt_in),
                "xtq": np.ascontiguousarray(cols),
                "wq": wq16, "wk": wk16, "wv": wv16,
                "mask": masks[r], "ones": ones,
            })
    return in_maps


def kernel(x, Wq, Wk, Wv):
    x = np.asarray(x, np.float32)
    Wq = np.asarray(Wq, np.float32)
    Wk = np.asarray(Wk, np.float32)
    Wv = np.asarray(Wv, np.float32)

    from concourse.bass_utils import run_bass_kernel_spmd

    nc = _get_nc()
    in_maps = build_in_maps(x, Wq, Wk, Wv)

    res = run_bass_kernel_spmd(nc, in_maps, core_ids=list(range(8)), trace=False)

    out = np.empty((B, S, D), np.float32)
    for b in range(B):
        for r in (0, 1):
            o = res.results[2 * b + r]["o"]
            for p, g in enumerate(QSETS[r]):
                out[b, g * P:(g + 1) * P, :] = o[p * P:(p + 1) * P, :]
    return out


# ---------------------------------------------------------------------------
# benchmarking support (not used by the grading path)

def _make_executor(nc, n_cores=8):
    """Build a cached jitted SPMD callable (no donation, reusable buffers)."""
    import jax
    from jax.sharding import Mesh, PartitionSpec
    try:
        from jax.experimental.shard_map import shard_map
    except ImportError:
        from jax.shard_map import shard_map
    from concourse import bass2jax
    from concourse import mybir as mb

    bass2jax.install_neuronx_cc_hook()
    partition_name = nc.partition_id_tensor.name if nc.partition_id_tensor else None
    in_names, out_names, out_avals, zero_outs = [], [], [], []
    for alloc in nc.m.functions[0].allocations:
        if not isinstance(alloc, mb.MemoryLocationSet):
            continue
        name = alloc.memorylocations[0].name
        if alloc.kind == "ExternalInput":
            if name != partition_name:
                in_names.append(name)
        elif alloc.kind == "ExternalOutput":
            shape = tuple(alloc.tensor_shape)
            dtype = mb.dt.np(alloc.dtype)
            out_names.append(name)
            out_avals.append(jax.core.ShapedArray(shape, dtype))
            zero_outs.append(np.zeros(shape, dtype))
    n_params = len(in_names)
    all_names = list(in_names) + list(out_names)
    if partition_name is not None:
        all_names.append(partition_name)

    def _body(*args):
        operands = list(args)
        if partition_name is not None:
            operands.append(bass2jax.partition_id_tensor())
        outs = bass2jax._bass_exec_p.bind(
            *operands,
            out_avals=tuple(out_avals),
            in_names=tuple(all_names),
            out_names=tuple(out_names),
            lowering_input_output_aliases=(),
            sim_require_finite=True,
            sim_require_nnan=True,
            nc=nc,
        )
        return tuple(outs)

    devices = jax.devices()[:n_cores]
    mesh = Mesh(np.asarray(devices), ("core",))
    in_specs = (PartitionSpec("core"),) * (n_params + len(out_names))
    out_specs = (PartitionSpec("core"),) * len(out_names)
    sharded = jax.jit(
        shard_map(_body, mesh=mesh, in_specs=in_specs, out_specs=out_specs,
                  check_rep=False),
        keep_unused=True,
    )
    return sharded, in_names, out_names, out_avals, zero_outs


def measure_exec_ns(iters=16, reps_pair=(2, 10), collective=None):
    """Estimate true per-core HW execution time of one kernel body.

    Per-call wall time through the axon tunnel is dominated by a transfer
    floor proportional to I/O bytes that completely hides execution. So we
    build NEFFs with the body repeated r1/r2 times (same I/O footprint) and
    use the slope: (wall(r2) - wall(r1)) / (r2 - r1). Collective-bearing
    NEFFs desync the mesh above ~reps 10, so keep reps small and iters high.
    """
    import time as _time
    import jax

    collective = COLLECTIVE if collective is None else collective
    rng = np.random.default_rng(0)
    x = rng.standard_normal((B, S, D)).astype(np.float32)
    sc = 1.0 / np.sqrt(D)
    Wq = rng.uniform(-sc, sc, (D, D)).astype(np.float32)
    Wk = rng.uniform(-sc, sc, (D, D)).astype(np.float32)
    Wv = rng.uniform(-sc, sc, (D, D)).astype(np.float32)
    in_maps = build_in_maps(x, Wq, Wk, Wv, collective=collective)

    pers = {}
    for reps in reps_pair:
        nc = build_nc(reps=reps, collective=collective)
        nc.finalize()
        sharded, in_names, out_names, out_avals, zero_outs = _make_executor(nc, 8)
        concat_in = [
            np.concatenate([np.asarray(in_maps[c][n]) for c in range(8)], axis=0)
            for n in in_names
        ]
        concat_zeros = [
            np.zeros((8 * z.shape[0], *z.shape[1:]), z.dtype) for z in zero_outs
        ]
        args = [jax.device_put(a) for a in concat_in + concat_zeros]
        jax.block_until_ready(args)
        outs = sharded(*args)
        jax.block_until_ready(outs)
        best = None
        for _ in range(5):
            t0 = _time.time()
            for _ in range(iters):
                outs = sharded(*args)
            jax.block_until_ready(outs)
            per = (_time.time() - t0) / iters
            best = per if best is None else min(best, per)
        pers[reps] = best
    r1, r2 = reps_pair
    return int((pers[r2] - pers[r1]) / (r2 - r1) * 1e9)
